# revision 1
# baseline (speedup 1.0000x reference)
"""Trainium2 Bass kernel for nn_DecoderStack (self-attn + cross-attn +
2-layer GELU FFN, shared decoder LN), 8-core data-parallel.

Sharding: 8 cores = 4 batches x 2 query-halves. Core c handles batch b=c//2,
query half h=c%2 (1024 tokens); K/V context is the full 2048 tokens of its
batch element (inputs only; no collectives).

Math restructuring (exact, up to float32r rounding):
  * softmax is invariant to the K-bias term, so  scores.T = x_kvT @ P  with
    P = (wq @ wk.T).T @ q_in + (wk @ bq)  — a single 1024-token projection
    replaces Q-proj and the 2048-token K-proj (host precomputes wq@wk.T).
  * PV is reassociated:  U = wv.T @ G + bv*denom,  G = x_tok.T-contraction
    of E  — the 2048-token V-proj becomes a 1024-token projection of G.

Layout: activations feature-major [D, S] (D on partitions); all matmuls in
float32r (TF32-like, full PE rate); scores transposed [t, s]; softmax
denominator via ones-column matmuls; LN stats via all-ones [128,128]
stationary matmuls whose sums land replicated on every partition. The
intermediate G stays in SBUF; only X1/X2 round-trip through DRAM.
"""
import sys
for _p in ("/opt/trn_rl_repo", "/root/.axon_site/_ro/trn_rl_repo"):
    if _p not in sys.path:
        sys.path.append(_p)

import numpy as np

import concourse.bass as bass
import concourse.tile as tile
from concourse import bacc, mybir
from concourse.bass_utils import run_bass_kernel_spmd

f32 = mybir.dt.float32
f32r = mybir.dt.float32r
AF = mybir.ActivationFunctionType
ALU = mybir.AluOpType

N_CORES = 8
B, S, T, D = 4, 2048, 2048, 1024
SH = S // 2          # per-core query tokens
KD = D // 128        # 8 d-tiles
TM = T // 128        # 16 t-tiles
SCALE = 1.0 / 8.0
LN_EPS = 1e-5
LN_RD = 1.0 / D

COLS = ["cp1", "cp2", "bv1", "bv2", "gm1", "bm1", "gm2", "bm2",
        "gd", "bd", "fb0", "fb1"]
NCOL = len(COLS)
ONES_COL = NCOL * 8  # last column of the packed cols input


def build_decoder(nc, taps=False, reps=0):
    """Emit the full per-core decoder program. Returns tap tensor names."""
    def din(name, shape, dt=f32r):
        return nc.dram_tensor(name, shape, dt, kind="ExternalInput").ap()

    xq = din("xq", [KD, 128, SH])          # x[b,half].T feature-major
    xkv = din("xkv", [KD, 128, T])         # x[b].T full (feature-major)
    ykv = din("ykv", [KD, 128, T])         # y[b].T full
    xtok = din("xtok", [TM, 128, D])       # x[b] token-major
    ytok = din("ytok", [TM, 128, D])       # y[b] token-major
    w = {n: din("w_" + n, [D, D]) for n in
         ["p1", "v1", "p2", "v2", "f0", "f1"]}
    cols_in = din("cols", [128, NCOL * 8 + 1 + 128], f32r)  # + ones col + ones128
    out = nc.dram_tensor("out", [KD, 128, SH], f32, kind="ExternalOutput").ap()

    tap_names = []

    with tile.TileContext(nc, pool_alloc_mode="queue") as tc:
        import contextlib
        rep_ctx = tc.For_i(0, reps, 1) if reps else contextlib.nullcontext()
        es = []

        def open_pool(name, bufs=1, space="SBUF"):
            cm = tc.tile_pool(name=name, bufs=bufs, space=space)
            pool = cm.__enter__()
            es.append(cm)
            return pool

        rep_ctx.__enter__()
        p_w = open_pool("w", bufs=2)          # weight halves [128,8,512] 16K
        p_st4 = open_pool("st4", bufs=3)      # [128,8,128] tile streams 4K
        p_stage = open_pool("stage", bufs=4)  # [128,1024] staging 4K
        p_bc = open_pool("bc", bufs=2)        # [128,1024] persist stats 4K
        p_rows = open_pool("rows", bufs=1)    # [1,1024] rows 4K
        p_cmn = open_pool("cmn", bufs=1)      # cols + ones128 4K
        p_act = open_pool("act", bufs=1)      # slotA 32K + slotB 64K
        p_dram = open_pool("dram", bufs=1, space="DRAM")
        p_psm = open_pool("psm", bufs=4, space="PSUM")   # [128,512]
        p_psr = open_pool("psr", bufs=4, space="PSUM")   # [128,512]

        cols_sb = p_cmn.tile([128, NCOL * 8 + 1 + 128], f32r, name="cols_sb")
        nc.sync.dma_start(cols_sb[:], cols_in)
        ones_sb = cols_sb[:, ONES_COL:ONES_COL + 1]      # [128,1] ones
        ones128 = cols_sb[:, ONES_COL + 1:ONES_COL + 129]  # [128,128] ones

        def col(name, j):
            c = COLS.index(name)
            return cols_sb[:, c * 8 + j: c * 8 + j + 1].bitcast(f32)

        def tap(name, ap_src, shape, dt_src=f32r):
            if not taps:
                return
            t = nc.dram_tensor("tap_" + name, shape, f32,
                               kind="ExternalOutput").ap()
            tap_names.append("tap_" + name)
            nc.sync.dma_start(t, ap_src.bitcast(f32) if dt_src == f32r else ap_src)

        def load_w_halves(wap):
            """Weight [D, D] as two halves [128, 8, 512] (d_out split)."""
            wr = wap.rearrange("(ko kp) d -> kp ko d", kp=128)
            halves = []
            for hf in range(2):
                t = p_w.tile([128, KD, 512], f32r, tag="w", name=f"wh{hf}")
                nc.sync.dma_start(t[:], wr[:, :, hf * 512:(hf + 1) * 512])
                halves.append(t)
            return halves

        def proj(out_write, wap, rhs_src, n_tok):
            """Feature-major projection: psum[m-tile, 512chunk] = w.T @ rhs.

            out_write(m, tch, ps): epilogue for the [128,512] PSUM tile.
            rhs_src: DRAM AP [KD, 128, n_tok] or SBUF tile [128, KD, n_tok].
            """
            wh = load_w_halves(wap)
            nch = n_tok // 512
            from_dram = rhs_src.space == bass.MemorySpace.DRAM
            pk_cm = tc.tile_pool(name="kvch", bufs=2)
            pk = pk_cm.__enter__()
            for tch in range(nch):
                sl = slice(tch * 512, (tch + 1) * 512)
                if from_dram:
                    kvc = pk.tile([128, KD, 512], f32r, tag="kv", name="kvc")
                    nc.sync.dma_start(
                        kvc[:], rhs_src[:, :, sl].rearrange("ko p s -> p ko s"))
                    rhs = lambda k: kvc[:, k, :]
                else:
                    rhs = lambda k, sl=sl: rhs_src[:, k, sl]
                for m in range(KD):
                    ps = p_psm.tile([128, 512], f32, tag="mm", name="proj_ps")
                    whf = wh[m // 4]
                    ml = m % 4
                    for k in range(KD):
                        nc.tensor.matmul(
                            ps[:], lhsT=whf[:, k, ml * 128:(ml + 1) * 128],
                            rhs=rhs(k), start=(k == 0), stop=(k == KD - 1))
                    out_write(m, tch, ps)
            pk_cm.__exit__(None, None, None)

        def layernorm(z_sb, g_cb, b_cb, out_cb):
            """LN over the feature dim (128 partitions x KD) of [128,KD,SH].

            Stats matmuls use an all-ones [128,128] stationary so sums land
            replicated on every partition (no broadcast step needed).
            """
            ps_s = [p_psr.tile([128, 512], f32, tag="row", name=f"lns{i}")
                    for i in range(2)]
            ps_q = [p_psr.tile([128, 512], f32, tag="row", name=f"lnq{i}")
                    for i in range(2)]
            for m in range(KD):
                sq = p_stage.tile([128, 1024], f32r, tag="stage", name="lnsq")
                nc.scalar.activation(sq[:], z_sb[:, m, :], AF.Square)
                for sch in range(2):
                    sl = slice(sch * 512, (sch + 1) * 512)
                    nc.tensor.matmul(ps_s[sch][:], lhsT=ones128[:],
                                     rhs=z_sb[:, m, sl],
                                     start=(m == 0), stop=(m == KD - 1))
                    nc.tensor.matmul(ps_q[sch][:], lhsT=ones128[:],
                                     rhs=sq[:, sl],
                                     start=(m == 0), stop=(m == KD - 1))
            mean = p_stage.tile([128, 1024], f32, tag="stage", name="mean")
            vp = p_stage.tile([128, 1024], f32, tag="stage", name="vp")
            for sch in range(2):
                sl = slice(sch * 512, (sch + 1) * 512)
                nc.vector.tensor_scalar(mean[:, sl], ps_s[sch][:], LN_RD, None,
                                        op0=ALU.mult)
                nc.vector.tensor_scalar(vp[:, sl], ps_q[sch][:], LN_RD, LN_EPS,
                                        op0=ALU.mult, op1=ALU.add)
            msq = p_stage.tile([128, 1024], f32, tag="stage", name="msq")
            nc.vector.tensor_mul(msq[:], mean[:], mean[:])
            varc = p_stage.tile([128, 1024], f32, tag="stage", name="varc")
            nc.vector.tensor_sub(varc[:], vp[:], msq[:])
            std = p_stage.tile([128, 1024], f32, tag="stage", name="std")
            nc.scalar.activation(std[:], varc[:], AF.Sqrt)
            rstd = p_bc.tile([128, 1024], f32, tag="bc", name="rstd")
            nc.vector.reciprocal(rstd[:], std[:])
            cr = p_bc.tile([128, 1024], f32, tag="bc", name="cr")
            nc.vector.tensor_mul(cr[:], mean[:], rstd[:])
            for m in range(KD):
                t1 = p_stage.tile([128, 1024], f32, tag="stage", name="ln_t1")
                nc.vector.tensor_mul(t1[:], z_sb[:, m, :].bitcast(f32), rstd[:])
                t2 = p_stage.tile([128, 1024], f32, tag="stage", name="ln_t2")
                nc.vector.tensor_sub(t2[:], t1[:], cr[:])
                nc.vector.tensor_scalar(out_cb(m), t2[:], g_cb(m), b_cb(m),
                                        op0=ALU.mult, op1=ALU.add)

        def attention_block(qin_d, kvF_d, kvT_d, wP, wV, cpn, bvn,
                            gmn, bmn, xout_d, blk):
            # ---- P projection (slot A): P = wP.T @ qin + cp ----
            p_sb = p_act.tile([128, KD, SH], f32r, tag="slotA", name="p_sb")

            def pwrite(m, sch, ps):
                nc.vector.tensor_scalar(p_sb[:, m, sch * 512:(sch + 1) * 512],
                                        ps[:], col(cpn, m), None, op0=ALU.add)
            proj(pwrite, wP, qin_d, SH)
            tap(f"P{blk}", p_sb[:], [128, KD, SH])

            # ---- scores.T = kvF.T-contraction of P ; exp ; denominator ----
            e_sb = p_act.tile([128, TM, SH], f32r, tag="slotB", name="e_sb")
            for tm in range(TM):
                kt = p_st4.tile([128, KD, 128], f32r, tag="st4", name="kt")
                nc.sync.dma_start(
                    kt[:], kvF_d[:, :, tm * 128:(tm + 1) * 128]
                    .rearrange("ko p t -> p ko t"))
                for sch in range(2):
                    sl = slice(sch * 512, (sch + 1) * 512)
                    ps = p_psm.tile([128, 512], f32, tag="mm", name="sc_ps")
                    for k in range(KD):
                        nc.tensor.matmul(ps[:], lhsT=kt[:, k, :],
                                         rhs=p_sb[:, k, sl],
                                         start=(k == 0), stop=(k == KD - 1))
                    nc.scalar.activation(e_sb[:, tm, sl], ps[:], AF.Exp,
                                         scale=SCALE)

            # ---- G = kvT.T-contraction of E (slot A); denom rides along ----
            g_sb = p_act.tile([128, KD, SH], f32r, tag="slotA", name="g_sb")
            ps_d = [p_psr.tile([1, 512], f32, tag="row", name=f"dn{i}")
                    for i in range(2)]
            for tm in range(TM):
                for sch in range(2):
                    sl = slice(sch * 512, (sch + 1) * 512)
                    nc.tensor.matmul(ps_d[sch][:], lhsT=ones_sb,
                                     rhs=e_sb[:, tm, sl],
                                     start=(tm == 0), stop=(tm == TM - 1))
            rden_row = p_rows.tile([1, 1024], f32, tag="row", name="rden_row")
            for sch in range(2):
                sl = slice(sch * 512, (sch + 1) * 512)
                nc.vector.reciprocal(rden_row[:, sl], ps_d[sch][:])
            if taps:
                den_r = p_rows.tile([1, 1024], f32, tag="row", name="den_r")
                for sch in range(2):
                    nc.scalar.copy(den_r[:, sch * 512:(sch + 1) * 512],
                                   ps_d[sch][:])
                tap(f"den{blk}", den_r[:], [1, 1024], f32)
            rden_bc = p_bc.tile([128, 1024], f32, tag="bc", name="rden_bc")
            nc.gpsimd.partition_broadcast(rden_bc[:], rden_row[:])
            for m in range(KD):
                vh = []
                for hfm in range(2):
                    vt = p_st4.tile([128, 8, 128], f32r, tag="st4", name="vh")
                    nc.sync.dma_start(
                        vt[:], kvT_d[hfm * 8:(hfm + 1) * 8, :,
                                     m * 128:(m + 1) * 128]
                        .rearrange("tm p d -> p tm d"))
                    vh.append(vt)
                psu = [p_psm.tile([128, 512], f32, tag="mm", name=f"pv{i}")
                       for i in range(2)]
                for tm in range(TM):
                    vt = vh[tm // 8][:, tm % 8, :]
                    for sch in range(2):
                        sl = slice(sch * 512, (sch + 1) * 512)
                        nc.tensor.matmul(psu[sch][:], lhsT=vt,
                                         rhs=e_sb[:, tm, sl],
                                         start=(tm == 0), stop=(tm == TM - 1))
                for sch in range(2):
                    nc.scalar.copy(g_sb[:, m, sch * 512:(sch + 1) * 512],
                                   psu[sch][:])

            # ---- U = wV.T @ G ; normalize ; +bv ; +resid -> Z (slot B) ----
            z_sb = p_act.tile([128, KD, SH], f32r, tag="slotB", name="z_sb")

            def uwrite(m, sch, ps):
                sl = slice(sch * 512, (sch + 1) * 512)
                rt = p_stage.tile([128, 1024], f32r, tag="stage", name="res_t")
                nc.sync.dma_start(rt[:, 0:512], qin_d[m, :, sl])
                t1 = p_stage.tile([128, 1024], f32, tag="stage", name="pv_t1")
                nc.vector.tensor_mul(t1[:, 0:512], ps[:], rden_bc[:, sl])
                t2 = p_stage.tile([128, 1024], f32, tag="stage", name="pv_t2")
                nc.vector.tensor_add(t2[:, 0:512], t1[:, 0:512],
                                     rt[:, 0:512].bitcast(f32))
                nc.vector.tensor_scalar(z_sb[:, m, sl], t2[:, 0:512],
                                        col(bvn, m), None, op0=ALU.add)
            proj(uwrite, wV, g_sb, SH)
            tap(f"Z1_{blk}", z_sb[:], [128, KD, SH])

            # ---- LN_m (in-place) ; + resid (in-place) ; LN_d -> xout ----
            layernorm(z_sb, lambda m: col(gmn, m), lambda m: col(bmn, m),
                      lambda m: z_sb[:, m, :])
            for m in range(KD):
                for sch in range(2):
                    sl = slice(sch * 512, (sch + 1) * 512)
                    rt = p_stage.tile([128, 1024], f32r, tag="stage",
                                      name="res2_t")
                    nc.sync.dma_start(rt[:, 0:512], qin_d[m, :, sl])
                    nc.vector.tensor_add(z_sb[:, m, sl], z_sb[:, m, sl],
                                         rt[:, 0:512])
            sts = {}

            def xcb(m):
                st = p_stage.tile([128, 1024], f32r, tag="stage", name="xo_st")
                sts[m] = st
                return st[:, 0:SH]
            layernorm(z_sb, lambda m: col("gd", m), lambda m: col("bd", m), xcb)
            for m in range(KD):
                nc.sync.dma_start(xout_d[m, :, :], sts[m][:, 0:SH])

        # ================= decoder =================
        x1_d = p_dram.tile([KD, 128, SH], f32r, tag="x1", name="x1_d")
        attention_block(xq, xkv, xtok, w["p1"], w["v1"], "cp1", "bv1",
                        "gm1", "bm1", x1_d, 1)
        x2_d = p_dram.tile([KD, 128, SH], f32r, tag="x2", name="x2_d")
        attention_block(x1_d, ykv, ytok, w["p2"], w["v2"], "cp2", "bv2",
                        "gm2", "bm2", x2_d, 2)

        # ================= FFN =================
        h1 = p_act.tile([128, KD, SH], f32r, tag="slotA", name="h1")

        def h1w(m, sch, ps):
            nc.scalar.activation(h1[:, m, sch * 512:(sch + 1) * 512], ps[:],
                                 AF.Gelu, bias=col("fb0", m))
        proj(h1w, w["f0"], x2_d, SH)

        z5 = p_act.tile([128, KD, SH], f32r, tag="slotB", name="z5")

        def h2w(m, sch, ps):
            sl = slice(sch * 512, (sch + 1) * 512)
            t1 = p_stage.tile([128, 1024], f32, tag="stage", name="h2_t")
            nc.scalar.activation(t1[:, 0:512], ps[:], AF.Gelu,
                                 bias=col("fb1", m))
            rt = p_stage.tile([128, 1024], f32r, tag="stage", name="resf_t")
            nc.sync.dma_start(rt[:, 0:512], x2_d[m, :, sl])
            nc.vector.tensor_add(z5[:, m, sl], t1[:, 0:512],
                                 rt[:, 0:512].bitcast(f32))
        proj(h2w, w["f1"], h1, SH)

        outs = {}

        def out_cb(m):
            st = p_stage.tile([128, 1024], f32r, tag="stage", name="out_st")
            outs[m] = st
            return st[:, 0:SH]
        layernorm(z5, lambda m: col("gd", m), lambda m: col("bd", m), out_cb)
        for m in range(KD):
            nc.sync.dma_start(out[m, :, :], outs[m][:, 0:SH].bitcast(f32))

        for cm in reversed(es):
            cm.__exit__(None, None, None)
        rep_ctx.__exit__(None, None, None)

    nc.compile()
    return tap_names


def _prep_inputs(inputs):
    """Host-side sharding + weight folding: returns in_maps (8 dicts)."""
    f64 = lambda k: np.asarray(inputs[k], np.float64)
    x, y = inputs["x"], inputs["y"]
    # folded attention weights: P = (wq@wk.T).T @ qin + wk@bq
    wp1 = (f64("wq_m") @ f64("wk_m").T).astype(np.float32)
    cp1 = (f64("wk_m") @ f64("bq_m")).astype(np.float32)
    wp2 = (f64("wq_c") @ f64("wk_c").T).astype(np.float32)
    cp2 = (f64("wk_c") @ f64("bq_c")).astype(np.float32)
    colvecs = {
        "cp1": cp1, "cp2": cp2,
        "bv1": inputs["bv_m"], "bv2": inputs["bv_c"],
        "gm1": inputs["g_m"], "bm1": inputs["b_m"],
        "gm2": inputs["g_c"], "bm2": inputs["b_c"],
        "gd": inputs["g_d"], "bd": inputs["b_d"],
        "fb0": inputs["f0_b"], "fb1": inputs["f1_b"],
    }
    cols = np.empty((128, NCOL * 8 + 1 + 128), np.float32)
    for c, n in enumerate(COLS):
        cols[:, c * 8:(c + 1) * 8] = np.asarray(colvecs[n], np.float32) \
            .reshape(KD, 128).T
    cols[:, ONES_COL:] = 1.0
    shared = {
        "w_p1": wp1, "w_p2": wp2,
        "w_v1": np.asarray(inputs["wv_m"], np.float32),
        "w_v2": np.asarray(inputs["wv_c"], np.float32),
        "w_f0": np.asarray(inputs["f0_w"], np.float32),
        "w_f1": np.asarray(inputs["f1_w"], np.float32),
        "cols": cols,
    }
    in_maps = []
    for c in range(N_CORES):
        b, h = c // 2, c % 2
        xb = np.asarray(x[b], np.float32)
        yb = np.asarray(y[b], np.float32)
        xT = np.ascontiguousarray(xb.T)  # [D, T]
        yT = np.ascontiguousarray(yb.T)
        m = dict(shared)
        m["xkv"] = xT.reshape(KD, 128, T)
        m["ykv"] = yT.reshape(KD, 128, T)
        m["xtok"] = np.ascontiguousarray(xb).reshape(TM, 128, D)
        m["ytok"] = np.ascontiguousarray(yb).reshape(TM, 128, D)
        m["xq"] = np.ascontiguousarray(
            xT[:, h * SH:(h + 1) * SH]).reshape(KD, 128, SH)
        in_maps.append(m)
    return in_maps


def kernel(**inputs):
    nc = bacc.Bacc("TRN2", target_bir_lowering=False, debug=False,
                   num_devices=N_CORES)
    build_decoder(nc, taps=False)
    in_maps = _prep_inputs(inputs)
    res = run_bass_kernel_spmd(nc, in_maps, core_ids=list(range(N_CORES)),
                               trace=False)
    out = np.empty((B, S, D), np.float32)
    for c in range(N_CORES):
        b, h = c // 2, c % 2
        o = res.results[c]["out"].reshape(D, SH)  # feature-major [d, s]
        out[b, h * SH:(h + 1) * SH, :] = o.T
    return out



# revision 33
# speedup vs baseline: 2.7668x; 2.7668x over previous
"""Trainium2 Bass kernel for nn_DecoderStack (self-attn + cross-attn +
2-layer GELU FFN, shared decoder LN), 8-core data-parallel.

Sharding: 8 cores = 4 batches x 2 query-halves. Core c handles batch b=c//2,
query half h=c%2 (1024 tokens); K/V context is the full 2048 tokens of its
batch element (inputs only; no collectives).

Math restructuring (exact up to rounding):
  * softmax K-bias invariance:  scores.T = x_kvT @ P  with
    P = (wq @ wk.T).T @ q_in + (wk @ bq)  (host precomputes wq@wk.T).
  * PV reassociation:  U = wv.T @ G,  G = x_tok.T-contraction of E.
  * E = exp(scores/8 - 9): the constant shift keeps E/G/U in fp16 range
    (softmax shift-invariance).
  * no reciprocal anywhere: LN is scale-invariant per token, so instead of
    U/den the residual is scaled UP:  z' = U + (qin + bv)*den, and the
    LN_m epsilon becomes eps*den^2.
  * rstd = exp(-0.5*ln(var)) on the scalar engine (single act table set).

All matmul operands are fp16 (10-bit mantissa ~ f32r precision): weights,
kv tiles, activations. PSUM accumulates fp32. Everything SBUF-resident
between blocks (no DRAM round-trips); x-chain (xq -> x1 -> x2) lives in one
slot, z'/z2/z5 in another, both updated in place tile-by-tile.
"""
import sys
for _p in ("/opt/trn_rl_repo", "/root/.axon_site/_ro/trn_rl_repo"):
    if _p not in sys.path:
        sys.path.append(_p)

import numpy as np

import concourse.bass as bass
import concourse.tile as tile
from concourse import bacc, mybir
from concourse.bass_utils import run_bass_kernel_spmd

f32 = mybir.dt.float32
fp16 = mybir.dt.float16
AF = mybir.ActivationFunctionType
ALU = mybir.AluOpType

N_CORES = 8
B, S, T, D = 4, 2048, 2048, 1024
SH = S // 2          # per-core query tokens
KD = D // 128        # 8 feature tiles
TM = T // 128        # 16 kv-token tiles
SCALE = 1.0 / 8.0
ESHIFT = -9.0        # exp(scores*SCALE + ESHIFT); softmax shift-invariant
LN_EPS = 1e-5
LN_RD = 1.0 / D

COLS = ["cp1", "cp2", "bv1", "bv2", "gm1", "bm1", "gm2", "bm2",
        "gd", "bd", "fb0", "fb1"]
NCOL = len(COLS)


def build_decoder(nc, taps=False, reps=0):
    """Emit the full per-core decoder program. Returns tap tensor names."""
    def din(name, shape, dt=fp16):
        return nc.dram_tensor(name, shape, dt, kind="ExternalInput").ap()

    # All inputs host-pre-arranged so every DMA is a dense per-partition copy
    # (strided gather descriptors cost ~0.8us each on the sync queue).
    xq = din("xq", [128, KD, SH])          # queries, partition-major
    xkv = din("xkv", [TM, 128, KD, 128])   # kt tiles per kv-token block
    ykv = din("ykv", [TM, 128, KD, 128])
    xtok = din("xtok", [KD, 128, TM, 128])  # vt tiles per feature block
    ytok = din("ytok", [KD, 128, TM, 128])
    w = {n: din("w_" + n, [2, 128, KD, 512]) for n in
         ["p1", "v1", "p2", "v2", "f0", "f1"]}
    cols_in = din("cols", [128, NCOL * 8], f32)
    out = nc.dram_tensor("out", [KD, 128, SH], fp16,
                         kind="ExternalOutput").ap()

    tap_names = []

    with tile.TileContext(nc, pool_alloc_mode="queue") as tc:
        import contextlib
        rep_ctx = tc.For_i(0, reps, 1) if reps else contextlib.nullcontext()
        es = []

        def open_pool(name, bufs=1, space="SBUF"):
            cm = tc.tile_pool(name=name, bufs=bufs, space=space)
            pool = cm.__enter__()
            es.append(cm)
            return pool

        rep_ctx.__enter__()
        p_w = open_pool("w", bufs=3)          # weight halves [128,KD,512]f16 8K
        p_st4 = open_pool("st4", bufs=3)      # kt/vt tiles [128,8,128]f16 2K
        p_stage = open_pool("stage", bufs=6)  # [128,1024] f16/f32 scratch
        p_row = open_pool("row", bufs=5)      # [128,1024] bc rows
        p_cmn = open_pool("cmn", bufs=1)      # cols + ones + const cols
        p_eb = open_pool("eb", bufs=1)        # e_sb 32K
        p_a = open_pool("a", bufs=1)          # slotA: p_sb/g_sb/h1 16K
        p_z = open_pool("z", bufs=1)          # slotZ: z'/z2/z5 16K
        p_q = open_pool("q", bufs=1)          # slotQ: xq/x1/x2 16K
        p_d = open_pool("d", bufs=1)          # slotD: qd 16K
        p_psm = open_pool("psm", bufs=4, space="PSUM")   # [128,512] mm outs
        p_psr = open_pool("psr", bufs=4, space="PSUM")   # [128,512] den/stats

        cols_sb = p_cmn.tile([128, NCOL * 8], f32, name="cols_sb")
        nc.sync.dma_start(cols_sb[:], cols_in)
        ones_h = p_cmn.tile([128, 128], fp16, name="ones_h")
        nc.vector.memset(ones_h[:], 1.0)
        esh_col = p_cmn.tile([128, 1], f32, name="esh_col")
        nc.vector.memset(esh_col[:], ESHIFT)

        def load_act_set(set_id):
            """Pre-place an ACT table load (greedy auto-insertion thrashes
            between exp_and_others and natural_log; set 6 has exp+ln+square
            +copy+identity, set 10 has gelu+square+copy+identity)."""
            tl = mybir.InstLoadActFuncSet(
                name=nc.get_next_instruction_name(),
                act_func_set_id=set_id, ins=[], outs=[])
            nc.scalar.add_instruction(tl)

        load_act_set(6)   # natural_log_exp_and_others

        def col(name, j):
            c = COLS.index(name)
            return cols_sb[:, c * 8 + j: c * 8 + j + 1]

        def tapf32(name, ap_src, shape):
            if not taps:
                return
            t = nc.dram_tensor("tap_" + name, shape, f32,
                               kind="ExternalOutput").ap()
            tap_names.append("tap_" + name)
            nc.sync.dma_start(t, ap_src)

        def tap16(name, tile16, shape):
            """Tap an fp16 SBUF tile (converted via ACT copy to f32)."""
            if not taps:
                return
            st = p_stage.tile([128, 1024], f32, tag="stagef", name="tapcv",
                              bufs=3)
            t = nc.dram_tensor("tap_" + name, shape, f32,
                               kind="ExternalOutput").ap()
            tap_names.append("tap_" + name)
            nc.scalar.activation(st[:, :shape[-1]], tile16, AF.Copy)
            nc.sync.dma_start(t, st[:, :shape[-1]])

        def load_w_halves(wap):
            """Weight pre-arranged by the host as [2, 128, KD, 512]."""
            halves = []
            for hf in range(2):
                t = p_w.tile([128, KD, 512], fp16, tag="w", name=f"wh{hf}")
                nc.sync.dma_start(t[:], wap[hf])
                halves.append(t)
            return halves

        def warm(dep_ap, n):
            """Keep the PE clock hot through a known idle window: n dummy
            matmuls gated on dep_ap (so they fire during the window, not
            before it)."""
            ps = p_psm.tile([128, 512], f32, tag="mm", name="warm_ps")
            for _ in range(n):
                nc.tensor.matmul(ps[:], lhsT=ones_h[:], rhs=dep_ap,
                                 start=True, stop=True)

        def proj(out_write, wh, rhs_sb, hooks=None):
            """out[m,tch] = w.T @ rhs for rhs [128, KD, SH] fp16 in SBUF.

            out_write(m, tch, ps): epilogue for the [128,512] PSUM tile.
            hooks: optional list of 16 callables/None, hooks[tch*8+m] emitted
            after slot (tch, m) — used to interleave a carried LN chain's
            DVE/ACT/stats work into this projection's matmul stream.
            """
            for tch in range(2):
                sl = slice(tch * 512, (tch + 1) * 512)
                for m in range(KD):
                    ps = p_psm.tile([128, 512], f32, tag="mm", name="proj_ps")
                    whf = wh[m // 4]
                    ml = m % 4
                    for k in range(KD):
                        nc.tensor.matmul(
                            ps[:], lhsT=whf[:, k, ml * 128:(ml + 1) * 128],
                            rhs=rhs_sb[:, k, sl],
                            start=(k == 0), stop=(k == KD - 1))
                    out_write(m, tch, ps)
                    if hooks is not None and hooks[tch * 8 + m] is not None:
                        hooks[tch * 8 + m]()

        def ln_rows(ps_s, ps_q, rstd_t, cr_t, tch, eps_row=None):
            """Row math for one 512-token chunk: mean/var -> rstd, cr.

            ps_s/ps_q: [128,512] PSUM sums of z and z^2 (replicated).
            rstd_t/cr_t: fp16 [128,1024] tiles, written at [:, tch*512:].
            eps_row: f32 [128,1024] eps*den^2 tile (LN_m) or None (+LN_EPS).
            """
            sl = slice(tch * 512, (tch + 1) * 512)
            mean = p_row.tile([128, 1024], fp16, tag="rowf", name="mean")
            nc.vector.tensor_scalar(mean[:, :512], ps_s[:], LN_RD, None,
                                    op0=ALU.mult)
            vp = p_row.tile([128, 1024], fp16, tag="rowf", name="vp")
            if eps_row is None:
                nc.vector.tensor_scalar(vp[:, :512], ps_q[:], LN_RD, LN_EPS,
                                        op0=ALU.mult, op1=ALU.add)
            else:
                nc.vector.scalar_tensor_tensor(vp[:, :512], ps_q[:], LN_RD,
                                               eps_row[:, sl], op0=ALU.mult,
                                               op1=ALU.add)
            m2 = p_row.tile([128, 1024], fp16, tag="rowf", name="m2")
            nc.vector.tensor_mul(m2[:, :512], mean[:, :512], mean[:, :512])
            var = p_row.tile([128, 1024], fp16, tag="rowf", name="var")
            nc.vector.tensor_sub(var[:, :512], vp[:, :512], m2[:, :512])
            lnv = p_row.tile([128, 1024], fp16, tag="rowf", name="lnv")
            nc.scalar.activation(lnv[:, :512], var[:, :512], AF.Ln)
            nc.scalar.activation(rstd_t[:, sl], lnv[:, :512], AF.Exp,
                                 scale=-0.5)
            nc.vector.tensor_mul(cr_t[:, sl], mean[:, :512], rstd_t[:, sl])

        def attention_block(qin, kvF, kvT, wPh, wVh, cpn, bvn,
                            gmn, bmn, blk, carry=None):
            """qin: slotQ [128,KD,SH] fp16 (also receives x_out in place).
            Writes x_out into qin slot tile-by-tile. `carry` is the previous
            phase's leftover LN-chain steps (8 closures), interleaved into
            this block's P projection. Returns this block's tch1 LN-chain
            steps for the next phase to interleave."""
            # ---- P projection (slot A): P = wP.T @ qin + cp ----
            p_sb = p_a.tile([128, KD, SH], fp16, tag="slotA", name="p_sb")

            def pwrite(m, tch, ps):
                nc.scalar.activation(p_sb[:, m, tch * 512:(tch + 1) * 512],
                                     ps[:], AF.Identity, bias=col(cpn, m))
            hooks = (list(carry) + [None] * 8) if carry else None
            proj(pwrite, wPh, qin, hooks=hooks)
            tap16(f"P{blk}_m0", p_sb[:, 0, :], [128, SH])

            # ---- scores.T; E = exp(s/8 - 9); den rides along ----
            e_sb = p_eb.tile([128, TM, SH], fp16, tag="eb", name="e_sb")
            ps_d = [p_psr.tile([128, 512], f32, tag="psr", name=f"dn{i}")
                    for i in range(2)]
            for tm in range(TM):
                kt = p_st4.tile([128, KD, 128], fp16, tag="st4", name="kt")
                nc.sync.dma_start(kt[:], kvF[tm])
                for sch in range(2):
                    sl = slice(sch * 512, (sch + 1) * 512)
                    ps = p_psm.tile([128, 512], f32, tag="mm", name="sc_ps")
                    for k in range(KD):
                        nc.tensor.matmul(ps[:], lhsT=kt[:, k, :],
                                         rhs=p_sb[:, k, sl],
                                         start=(k == 0), stop=(k == KD - 1))
                    nc.scalar.activation(e_sb[:, tm, sl], ps[:], AF.Exp,
                                         scale=SCALE, bias=esh_col[:])
                    nc.tensor.matmul(ps_d[sch][:], lhsT=ones_h[:],
                                     rhs=e_sb[:, tm, sl],
                                     start=(tm == 0), stop=(tm == TM - 1))

            # ---- den rows; qd = (qin + bv) * den (while G runs) ----
            den_bc = p_row.tile([128, 1024], fp16, tag="rowp", name="den_bc",
                                bufs=2)
            for sch in range(2):
                sl = slice(sch * 512, (sch + 1) * 512)
                nc.scalar.activation(den_bc[:, sl], ps_d[sch][:], AF.Copy)
            if taps:
                dtap = p_stage.tile([128, 1024], f32, tag="stagef",
                                    name="dtap", bufs=3)
                nc.scalar.activation(dtap[:], den_bc[:], AF.Copy)
                tapf32(f"den{blk}", dtap[:1, :], [1, 1024])
            eps_row = p_row.tile([128, 1024], fp16, tag="rowp", name="eps_row",
                                 bufs=2)
            nc.vector.scalar_tensor_tensor(eps_row[:], den_bc[:], LN_EPS,
                                           den_bc[:], op0=ALU.mult,
                                           op1=ALU.mult)
            qd = p_d.tile([128, KD, SH], fp16, tag="slotD", name="qd")
            for m in range(KD):
                qb = p_stage.tile([128, 1024], fp16, tag="stageh", name="qb")
                nc.vector.tensor_scalar(qb[:], qin[:, m, :], col(bvn, m),
                                        None, op0=ALU.add)
                nc.vector.tensor_mul(qd[:, m, :], qb[:], den_bc[:])

            # ---- G = kvT.T-contraction of E (slot A, replaces P) ----
            g_sb = p_a.tile([128, KD, SH], fp16, tag="slotA", name="g_sb")
            for m in range(KD):
                vt2 = p_st4.tile([128, TM, 128], fp16, tag="vt2", name="vt2",
                                 bufs=2)
                nc.sync.dma_start(vt2[:], kvT[m])
                psu = [p_psm.tile([128, 512], f32, tag="mm", name=f"pv{i}")
                       for i in range(2)]
                for tm in range(TM):
                    for sch in range(2):
                        sl = slice(sch * 512, (sch + 1) * 512)
                        nc.tensor.matmul(psu[sch][:], lhsT=vt2[:, tm, :],
                                         rhs=e_sb[:, tm, sl],
                                         start=(tm == 0), stop=(tm == TM - 1))
                for sch in range(2):
                    nc.scalar.activation(g_sb[:, m, sch * 512:(sch + 1) * 512],
                                         psu[sch][:], AF.Copy)

            # ---- U-proj + z' = U + qd ; LN_m stats per (tch, m) ----
            zp = p_z.tile([128, KD, SH], fp16, tag="slotZ", name="zp")
            st_m = {}

            def uwrite(m, tch, ps):
                sl = slice(tch * 512, (tch + 1) * 512)
                nc.vector.tensor_add(zp[:, m, sl], ps[:], qd[:, m, sl])
                sq = p_stage.tile([128, 1024], fp16, tag="stageh", name="sq")
                nc.scalar.activation(sq[:, :512], zp[:, m, sl], AF.Square)
                if tch not in st_m:
                    st_m[tch] = [
                        p_psr.tile([128, 512], f32, tag="psr", name="st_s"),
                        p_psr.tile([128, 512], f32, tag="psr", name="st_q")]
                nc.tensor.matmul(st_m[tch][0][:], lhsT=ones_h[:],
                                 rhs=zp[:, m, sl],
                                 start=(m == 0), stop=(m == KD - 1))
                nc.tensor.matmul(st_m[tch][1][:], lhsT=ones_h[:],
                                 rhs=sq[:, :512],
                                 start=(m == 0), stop=(m == KD - 1))
            # ---- LN chain for one 512-token chunk, split into 8 steps so
            #      it can interleave with a projection's matmul stream:
            #      rows_m -> apply_m(+resid)+stats_d -> rows_d -> apply_d ----
            r1 = p_row.tile([128, 1024], fp16, tag="rowh", name="r1", bufs=4)
            c1 = p_row.tile([128, 1024], fp16, tag="rowh", name="c1", bufs=4)
            r2 = p_row.tile([128, 1024], fp16, tag="rowh", name="r2", bufs=4)
            c2 = p_row.tile([128, 1024], fp16, tag="rowh", name="c2", bufs=4)

            def chain(tch):
                sl = slice(tch * 512, (tch + 1) * 512)
                st_d = []

                def apply_m(m):
                    # z2 = z'*(r1*g) + (qin - (c1*g - b)) [in place over z']
                    tmp = p_stage.tile([128, 1024], fp16, tag="stageh",
                                       name="tmp")
                    nc.vector.tensor_scalar(tmp[:, :512], c1[:, sl],
                                            col(gmn, m), col(bmn, m),
                                            op0=ALU.mult, op1=ALU.subtract)
                    gr = p_stage.tile([128, 1024], fp16, tag="stageh",
                                      name="gr")
                    nc.vector.tensor_scalar(gr[:, :512], r1[:, sl],
                                            col(gmn, m), None, op0=ALU.mult)
                    t1 = p_stage.tile([128, 1024], fp16, tag="stageh",
                                      name="t1")
                    nc.vector.tensor_mul(t1[:, :512], zp[:, m, sl],
                                         gr[:, :512])
                    tq = p_stage.tile([128, 1024], fp16, tag="stageh",
                                      name="tq")
                    nc.vector.tensor_sub(tq[:, :512], qin[:, m, sl],
                                         tmp[:, :512])
                    nc.vector.tensor_add(zp[:, m, sl], t1[:, :512],
                                         tq[:, :512])
                    sq = p_stage.tile([128, 1024], fp16, tag="stageh",
                                      name="sq2")
                    nc.scalar.activation(sq[:, :512], zp[:, m, sl], AF.Square)
                    nc.tensor.matmul(st_d[0][:], lhsT=ones_h[:],
                                     rhs=zp[:, m, sl],
                                     start=(m == 0), stop=(m == KD - 1))
                    nc.tensor.matmul(st_d[1][:], lhsT=ones_h[:],
                                     rhs=sq[:, :512],
                                     start=(m == 0), stop=(m == KD - 1))

                def apply_d(m):
                    # x = z2*(r2*gd) - (c2*gd - bd)  [in place over qin]
                    tmp = p_stage.tile([128, 1024], fp16, tag="stageh",
                                       name="tmp2")
                    nc.vector.tensor_scalar(tmp[:, :512], c2[:, sl],
                                            col("gd", m), col("bd", m),
                                            op0=ALU.mult, op1=ALU.subtract)
                    gr = p_stage.tile([128, 1024], fp16, tag="stageh",
                                      name="gr2")
                    nc.vector.tensor_scalar(gr[:, :512], r2[:, sl],
                                            col("gd", m), None, op0=ALU.mult)
                    t1 = p_stage.tile([128, 1024], fp16, tag="stageh",
                                      name="t3")
                    nc.vector.tensor_mul(t1[:, :512], zp[:, m, sl],
                                         gr[:, :512])
                    nc.vector.tensor_sub(qin[:, m, sl], t1[:, :512],
                                         tmp[:, :512])

                def s_rows_m():
                    ln_rows(st_m[tch][0], st_m[tch][1], r1, c1, tch,
                            eps_row=eps_row)
                    st_d.extend([
                        p_psr.tile([128, 512], f32, tag="psr", name="sd_s"),
                        p_psr.tile([128, 512], f32, tag="psr", name="sd_q")])

                def s_rows_d():
                    ln_rows(st_d[0], st_d[1], r2, c2, tch)

                def pair(f, ms):
                    return lambda: [f(m) for m in ms]
                return [s_rows_m, pair(apply_m, [0, 1]), pair(apply_m, [2, 3]),
                        pair(apply_m, [4, 5]), pair(apply_m, [6, 7]),
                        s_rows_d, pair(apply_d, [0, 1, 2, 3]),
                        pair(apply_d, [4, 5, 6, 7])]

            # tch0's chain rides inside the U projection's tch1 slots.
            proj(uwrite, wVh, g_sb, hooks=[None] * 8 + chain(0))
            tap16(f"Z1_{blk}_m0", zp[:, 0, :], [128, SH])
            return chain(1)

        # ================= decoder =================
        # 2-way split on the critical first transfers: the first proj matmul
        # group only needs the front halves.
        qin = p_q.tile([128, KD, SH], fp16, tag="slotQ", name="qin")
        nc.sync.dma_start(qin[:, :KD // 2, :], xq[:, :KD // 2, :])
        wP1h = []
        for hf in range(2):
            t = p_w.tile([128, KD, 512], fp16, tag="w", name=f"wp1h{hf}")
            nc.sync.dma_start(t[:, :KD // 2, :], w["p1"][hf, :, :KD // 2, :])
            wP1h.append(t)
        nc.sync.dma_start(qin[:, KD // 2:, :], xq[:, KD // 2:, :])
        for hf in range(2):
            nc.sync.dma_start(wP1h[hf][:, KD // 2:, :],
                              w["p1"][hf, :, KD // 2:, :])
        wV1h = load_w_halves(w["v1"])
        c1rem = attention_block(qin, xkv, xtok, wP1h, wV1h, "cp1", "bv1",
                                "gm1", "bm1", 1)
        wP2h = load_w_halves(w["p2"])
        wV2h = load_w_halves(w["v2"])
        c2rem = attention_block(qin, ykv, ytok, wP2h, wV2h, "cp2", "bv2",
                                "gm2", "bm2", 2, carry=c1rem)

        # ================= FFN (qin now holds x2) =================
        # block2's leftover chain still uses ln/exp (act set 6), which
        # conflicts with gelu's table set — run it before the gelu load.
        for step in c2rem:
            step()
        wF0h = load_w_halves(w["f0"])
        wF1h = load_w_halves(w["f1"])
        load_act_set(10)  # gelu_and_others (square/copy/identity included)
        h1 = p_a.tile([128, KD, SH], fp16, tag="slotA", name="h1")

        def h1w(m, tch, ps):
            nc.scalar.activation(h1[:, m, tch * 512:(tch + 1) * 512], ps[:],
                                 AF.Gelu, bias=col("fb0", m))
        proj(h1w, wF0h, qin)

        z5 = p_z.tile([128, KD, SH], fp16, tag="slotZ", name="z5")
        st_f = {}

        def h2w(m, tch, ps):
            sl = slice(tch * 512, (tch + 1) * 512)
            t1 = p_stage.tile([128, 1024], fp16, tag="stageh", name="h2_t")
            nc.scalar.activation(t1[:, :512], ps[:], AF.Gelu,
                                 bias=col("fb1", m))
            nc.vector.tensor_add(z5[:, m, sl], t1[:, :512], qin[:, m, sl])
            sq = p_stage.tile([128, 1024], fp16, tag="stageh", name="sqf")
            nc.scalar.activation(sq[:, :512], z5[:, m, sl], AF.Square)
            if tch not in st_f:
                st_f[tch] = [
                    p_psr.tile([128, 512], f32, tag="psr", name="sf_s"),
                    p_psr.tile([128, 512], f32, tag="psr", name="sf_q")]
            nc.tensor.matmul(st_f[tch][0][:], lhsT=ones_h[:], rhs=z5[:, m, sl],
                             start=(m == 0), stop=(m == KD - 1))
            nc.tensor.matmul(st_f[tch][1][:], lhsT=ones_h[:], rhs=sq[:, :512],
                             start=(m == 0), stop=(m == KD - 1))
        proj(h2w, wF1h, h1)
        load_act_set(6)   # back for the final LN's ln/exp

        r3 = p_row.tile([128, 1024], fp16, tag="rowh", name="r3", bufs=4)
        c3 = p_row.tile([128, 1024], fp16, tag="rowh", name="c3", bufs=4)
        for tch in range(2):
            sl = slice(tch * 512, (tch + 1) * 512)
            ln_rows(st_f[tch][0], st_f[tch][1], r3, c3, tch)
            for m in range(KD):
                tmp = p_stage.tile([128, 1024], fp16, tag="stageh",
                                   name="tmpo")
                nc.vector.tensor_scalar(tmp[:, :512], c3[:, sl],
                                        col("gd", m), col("bd", m),
                                        op0=ALU.mult, op1=ALU.subtract)
                gr = p_stage.tile([128, 1024], fp16, tag="stageh", name="gro")
                nc.vector.tensor_scalar(gr[:, :512], r3[:, sl],
                                        col("gd", m), None, op0=ALU.mult)
                t1 = p_stage.tile([128, 1024], fp16, tag="stageh", name="t1o")
                nc.vector.tensor_mul(t1[:, :512], z5[:, m, sl], gr[:, :512])
                ot = p_stage.tile([128, 1024], fp16, tag="stageo", name="oto",
                                  bufs=8)
                nc.vector.tensor_sub(ot[:, :512], t1[:, :512], tmp[:, :512])
                nc.sync.dma_start(out[m, :, sl], ot[:, :512])

        for cm in reversed(es):
            cm.__exit__(None, None, None)
        rep_ctx.__exit__(None, None, None)

    nc.compile()
    return tap_names


def _prep_inputs(inputs):
    """Host-side sharding + weight folding: returns in_maps (8 dicts)."""
    f64 = lambda k: np.asarray(inputs[k], np.float64)
    x, y = np.asarray(inputs["x"], np.float32), np.asarray(inputs["y"],
                                                           np.float32)
    # folded attention weights: P = (wq@wk.T).T @ qin + wk@bq
    wp1 = (f64("wq_m") @ f64("wk_m").T).astype(np.float16)
    cp1 = (f64("wk_m") @ f64("bq_m")).astype(np.float32)
    wp2 = (f64("wq_c") @ f64("wk_c").T).astype(np.float16)
    cp2 = (f64("wk_c") @ f64("bq_c")).astype(np.float32)
    colvecs = {
        "cp1": cp1, "cp2": cp2,
        "bv1": inputs["bv_m"], "bv2": inputs["bv_c"],
        "gm1": inputs["g_m"], "bm1": inputs["b_m"],
        "gm2": inputs["g_c"], "bm2": inputs["b_c"],
        "gd": inputs["g_d"], "bd": inputs["b_d"],
        "fb0": inputs["f0_b"], "fb1": inputs["f1_b"],
    }
    cols = np.empty((128, NCOL * 8), np.float32)
    for c, n in enumerate(COLS):
        cols[:, c * 8:(c + 1) * 8] = np.asarray(colvecs[n], np.float32) \
            .reshape(KD, 128).T
    def warr(w16):
        """[D, D] -> [2, 128, KD, 512]: per-half, partition-major k-tiles."""
        wr = w16.reshape(KD, 128, D)
        return np.stack([
            np.ascontiguousarray(wr[:, :, hf * 512:(hf + 1) * 512]
                                 .transpose(1, 0, 2))
            for hf in range(2)])

    def kt_tiles(xT16):
        """[D, T] feature-major -> [TM, 128, KD, 128] dense kt tiles."""
        return np.ascontiguousarray(
            xT16.reshape(KD, 128, TM, 128).transpose(2, 1, 0, 3))

    def vt_tiles(x16):
        """[T, D] token-major -> [KD, 128, TM, 128] dense vt tiles."""
        return np.ascontiguousarray(
            x16.reshape(TM, 128, KD, 128).transpose(2, 1, 0, 3))

    shared = {
        "w_p1": warr(wp1), "w_p2": warr(wp2),
        "w_v1": warr(np.asarray(inputs["wv_m"], np.float16)),
        "w_v2": warr(np.asarray(inputs["wv_c"], np.float16)),
        "w_f0": warr(np.asarray(inputs["f0_w"], np.float16)),
        "w_f1": warr(np.asarray(inputs["f1_w"], np.float16)),
        "cols": cols,
    }
    in_maps = []
    for c in range(N_CORES):
        b, h = c // 2, c % 2
        xb16 = x[b].astype(np.float16)
        yb16 = y[b].astype(np.float16)
        xT = np.ascontiguousarray(xb16.T)  # [D, T]
        yT = np.ascontiguousarray(yb16.T)
        m = dict(shared)
        m["xkv"] = kt_tiles(xT)
        m["ykv"] = kt_tiles(yT)
        m["xtok"] = vt_tiles(xb16)
        m["ytok"] = vt_tiles(yb16)
        m["xq"] = np.ascontiguousarray(
            xT[:, h * SH:(h + 1) * SH].reshape(KD, 128, SH).transpose(1, 0, 2))
        in_maps.append(m)
    return in_maps


def kernel(**inputs):
    nc = bacc.Bacc("TRN2", target_bir_lowering=False, debug=False,
                   num_devices=N_CORES)
    build_decoder(nc, taps=False)
    in_maps = _prep_inputs(inputs)
    res = run_bass_kernel_spmd(nc, in_maps, core_ids=list(range(N_CORES)),
                               trace=False)
    out = np.empty((B, S, D), np.float32)
    for c in range(N_CORES):
        b, h = c // 2, c % 2
        o = res.results[c]["out"].astype(np.float32).reshape(D, SH)
        out[b, h * SH:(h + 1) * SH, :] = o.T
    return out
